# revision 42
# baseline (speedup 1.0000x reference)
"""Trainium2 Bass kernel for nn_Decoder_offset001 (dense CNN decoder with
deformable convs), data-parallel over 8 NeuronCores.

Sharding: 8 shards = 2 batches x 4 H-strips of 64 output rows, each strip
carrying a 14-row halo (92 rows, zero-padded at image borders) and 1-col
zero pads (258 wide).  Each core runs the full network on its strip; host
gathers the central 64 rows.

v2 design (f32r/bf16, 2-row batching):
  - All front convs use f32r (1 cy/row on the PE for N>=256) with
    block-diagonal [128,128] weights (slab0+slab1 in ONE matmul, no
    tile_position -- f32r forbids nonzero tile positions).  N=512 (2 rows).
  - Deform convs: exact bilinear with |d|<1, horizontal-first separable
    (all weights evaluated at the OUTPUT pixel):
      h_r(j,i) = X(j+r,1+i) + relu(dx)*DX(j+r,1+i) + min(dx,0)*DX(j+r,i)
      s(j,i)   = h_0 + relu(dy)*(h_1-h_0) + min(dy,0)*(h_0-h_-1)
    with DX(y,x) = X(y,x+1)-X(y,x) shared across kernel points and rows.
    Only dy/dx per point are broadcast (stream_shuffle) -- 2.25x less
    broadcast than the 81-product scheme; relu folds into
    scalar_tensor_tensor (DVE).
    Product chain in bf16 (all writes >=1.5KB contiguous), matmuls with
    block-diag bf16 weights (4 quarters in one matmul, N=512).
  - l15/l24 stay fp32(+tile_position) over bitcast rhs (small).
  - Per-row masked activations (f32r, 1KB writes) keep the SPMD border
    handling of v1.  NOTE: sub-1KB-per-partition engine writes to SBUF
    (e.g. 256-wide bf16 stores) catastrophically stall PE streaming on
    this HW -- every engine store here is >=1KB or >=1.5KB-contiguous.
"""
import sys
import numpy as np
import ml_dtypes

BF16 = ml_dtypes.bfloat16

for _p in ('/opt/trn_rl_repo',):
    if _p not in sys.path:
        sys.path.insert(0, _p)

RATIO = 0.08
GX = np.repeat(np.arange(-1, 2), 3)
GY = np.tile(np.arange(-1, 2), 3)
RC = [(r, c) for r in (-1, 0, 1) for c in (-1, 0, 1)]

HALO = 14
ROWS = 92
W = 256
WP = 258
SR = 52                  # 64-ch slab rows (local 0..51)
SOFF = 40                # slab1 strip-row offset
FROWS = 50               # front conv out-rows local 1..50 -> 25 pairs
QR = 28                  # 32-ch quarter rows
QOFF = [8 + 16 * g for g in range(4)]
NSTEP = 26               # deform out-rows local 1..26 -> 13 pairs
NP = 13
XRN = 6                  # x input ring slots
URN = 5                  # u ring rotation (tile has URN+1 slots, last = zero)
RRN = 6                  # relu ring rotation (slot RRN = zero)

_cache = {}


def split_excess_waits(nc, mybir):
    """Walrus allows 1 sync-wait per instruction (2 for EventSemaphore);
    Tile emits more.  Move excess waits onto inserted same-engine NOPs."""
    n = 0
    for bbh in nc.bb_map.values():
        bb = bbh.bb
        out, changed = [], False
        for inst in bb.instructions:
            si = inst.sync_info
            cap = 2 if isinstance(inst, mybir.InstEventSemaphore) else 1
            if si is not None and si.on_wait is not None and len(si.on_wait) > cap:
                waits = list(si.on_wait)
                extra, keep = waits[:-cap], waits[-cap:]
                for w_ in extra:
                    nop = mybir.InstNoOp(
                        name=nc.get_next_instruction_name(),
                        engine=inst.engine, ins=[], outs=[],
                        sync_info=mybir.SyncInfo(on_wait=[w_], on_update=[]))
                    nc.register_instruction(nop)
                    out.append(nop)
                    n += 1
                inst.sync_info = mybir.SyncInfo(on_wait=keep,
                                                on_update=si.on_update)
                changed = True
            out.append(inst)
        if changed:
            bb.instructions = out
    return n


def build_nc():
    import concourse.bass as bass
    import concourse.mybir as mybir
    import concourse.tile as tile
    from contextlib import ExitStack

    f32 = mybir.dt.float32
    bf16 = mybir.dt.bfloat16
    f32r = mybir.dt.float32r
    AF = mybir.ActivationFunctionType
    ALU = mybir.AluOpType

    nc = bass.Bass()
    xin = nc.declare_dram_parameter("xin", [64, ROWS, WP], f32r, isOutput=False)
    fcp = nc.declare_dram_parameter("fcp", [NP, 128, 1024], bf16, isOutput=False)
    wcv = nc.declare_dram_parameter("wcv", [128, 5 * 9 * 128], f32r, isOutput=False)
    w15 = nc.declare_dram_parameter("w15", [128, 9 * 32], f32, isOutput=False)
    wdf = nc.declare_dram_parameter("wdf", [128, 4 * 9 * 128], bf16, isOutput=False)
    wds = nc.declare_dram_parameter("wds", [128, 4 * 128], f32r, isOutput=False)
    w24 = nc.declare_dram_parameter("w24", [128, 9 * 3], f32, isOutput=False)
    b24 = nc.declare_dram_parameter("b24", [128, 1], f32, isOutput=False)
    maskc = nc.declare_dram_parameter("maskc", [128, FROWS], f32, isOutput=False)
    mbiasc = nc.declare_dram_parameter("mbiasc", [128, 5 * FROWS], f32, isOutput=False)
    mq15 = nc.declare_dram_parameter("mq15", [128, QR], f32, isOutput=False)
    mb15 = nc.declare_dram_parameter("mb15", [128, QR], f32, isOutput=False)
    maskq = nc.declare_dram_parameter("maskq", [128, NSTEP], f32, isOutput=False)
    mbiasq = nc.declare_dram_parameter("mbiasq", [128, 4 * NSTEP], f32, isOutput=False)
    out = nc.declare_dram_parameter("out", [3, 64, W], f32, isOutput=True)
    import os as _os
    DBG = _os.environ.get('KDBG', '')
    if DBG:
        dbg = nc.declare_dram_parameter("dbg", [128, QR, W], f32,
                                        isOutput=True)

    with ExitStack() as ctx:
        tc = ctx.enter_context(tile.TileContext(nc))
        wp_ = ctx.enter_context(tc.tile_pool(name="w", bufs=1))
        big = ctx.enter_context(tc.tile_pool(name="big", bufs=1))
        qp = ctx.enter_context(tc.tile_pool(name="q", bufs=1))
        fr = ctx.enter_context(tc.tile_pool(name="fld", bufs=2))
        gp = ctx.enter_context(tc.tile_pool(name="g", bufs=1))
        sp = ctx.enter_context(tc.tile_pool(name="s", bufs=2))
        ppF = ctx.enter_context(tc.tile_pool(name="psF", bufs=4, space="PSUM"))
        ppD = ctx.enter_context(tc.tile_pool(name="psD", bufs=4, space="PSUM"))

        def psum_tile(pool, tag):
            return pool.tile([128, 512], f32, tag=tag, name=tag)

        def load(tag, param, cols, dt=f32):
            t = wp_.tile([128, cols], dt, tag=tag)
            nc.sync.dma_start(t[:], param[:, :])
            return t

        wcv_t = load("wcv", wcv, 5 * 9 * 128, f32r)
        w15_t = load("w15", w15, 9 * 32)
        wdf_t = load("wdf", wdf, 4 * 9 * 128, bf16)
        wds_t = load("wds", wds, 4 * 128, f32r)
        w24_t = load("w24", w24, 9 * 3)
        b24_t = load("b24", b24, 1)
        mkc_t = load("mkc", maskc, FROWS)
        mbc_t = load("mbc", mbiasc, 5 * FROWS)
        mq15_t = load("mq15t", mq15, QR)
        mb15_t = load("mb15t", mb15, QR)
        mkq_t = load("mkq", maskq, NSTEP)
        mbq_t = load("mbq", mbiasq, 4 * NSTEP)

        def wcv_ap(stage, k):
            return wcv_t[:, (stage * 9 + k) * 128:(stage * 9 + k + 1) * 128]

        def wdf_ap(d, k):
            return wdf_t[:, (d * 9 + k) * 128:(d * 9 + k + 1) * 128]

        # ---- x input ring (slot = row % XRN) ----
        xr = big.tile([128, XRN, WP], f32r, tag="xring")
        for s in range(4):
            nc.sync.dma_start(xr[0:64, s, :], xin[:, s, :])
            nc.sync.dma_start(xr[64:128, s, :], xin[:, SOFF + s, :])

        T = big.tile([128, SR, WP], f32r, tag="T")
        nc.gpsimd.memset(T[:, 0, :].bitcast(f32), 0.0)
        nc.gpsimd.memset(T[:, SR - 1, :].bitcast(f32), 0.0)
        nc.gpsimd.memset(T[:, 1:SR - 1, 0:1].bitcast(f32), 0.0)
        nc.gpsimd.memset(T[:, 0:SR - 1, WP - 1:WP].bitcast(f32), 0.0)
        u1 = big.tile([128, URN + 1, WP], f32r, tag="u1")
        nc.gpsimd.memset(u1[:].bitcast(f32), 0.0)
        u2 = big.tile([128, URN + 1, WP], f32r, tag="u2")
        nc.gpsimd.memset(u2[:].bitcast(f32), 0.0)

        # ---------------- generic 9-tap pair conv on the PE ----------------
        def mm_pair(ps, w_fn, src, m):
            """Accumulate the 9 taps of a 3x3 conv for output rows (m, m+1)
            into ps[:, 0:512] (row m -> cols 0:256, m+1 -> 256:512).
            src(row) -> (tile, slot) or None (zero row).  Splits a tap into
            per-row N=256 matmuls when the two source rows are not
            slot-contiguous or one is invalid."""
            def pairinfo(r):
                sa, sb = src(m + r), src(m + 1 + r)
                cont = (sa is not None and sb is not None
                        and sa[0] is sb[0] and sb[1] == sa[1] + 1)
                return sa, sb, cont
            rstar = None
            for r in (0, -1, 1):
                if pairinfo(r)[2]:
                    rstar = r
                    break
            assert rstar is not None, f"no contiguous tap pair at m={m}"
            order = [(rstar, -1)]
            order += [(r, c) for r in (-1, 0, 1) for c in (-1, 0, 1)
                      if (r, c) != (rstar, -1) and (r, c) != (rstar, 1)]
            order += [(rstar, 1)]
            nmm = len(order) - 1
            for idx, (r, c) in enumerate(order):
                k = (r + 1) * 3 + (c + 1)
                w_ap = w_fn(k)
                sa, sb, cont = pairinfo(r)
                st, sx = (idx == 0), (idx == nmm)
                if cont:
                    t_, sl = sa
                    nc.tensor.matmul(
                        ps[:, 0:512], w_ap,
                        t_[:, sl:sl + 2, 1 + c:1 + c + W],
                        start=st, stop=sx, skip_group_check=True)
                else:
                    for half, sv in ((0, sa), (1, sb)):
                        if sv is None:
                            continue
                        t_, sl = sv
                        nc.tensor.matmul(
                            ps[:, 256 * half:256 * half + 256], w_ap,
                            t_[:, sl, 1 + c:1 + c + W],
                            start=st, stop=sx, skip_group_check=True)

        # evictions -------------------------------------------------------
        def act2(dst0, dst1, ps, func, mb0, mk0, mb1, mk1):
            nc.scalar.activation(dst0, ps[:, 0:256], func, bias=mb0, scale=mk0)
            nc.scalar.activation(dst1, ps[:, 256:512], func, bias=mb1,
                                 scale=mk1)

        def evict_resid_pair(dst_pair, ps, mb0, mk0, mb1, mk1, eng):
            ev = sp.tile([128, 512], f32r, tag="ev")
            act2(ev[:, 0:256], ev[:, 256:512], ps, AF.Identity,
                 mb0, mk0, mb1, mk1)
            ev2 = ev[:].rearrange("p (a b) -> p a b", a=2)
            eng.tensor_tensor(dst_pair, ev2, dst_pair, ALU.add)

        # ---------------- front stack, fused wavefront, R=2 ----------------
        NPAIR = FROWS // 2   # 25

        def src_xr(row):
            return (xr, row % XRN) if 0 <= row <= SR - 1 else None

        def src_T(row):
            return (T, row)

        def mk_src_u(u):
            def f(row):
                return (u, row % URN) if 1 <= row <= FROWS else None
            return f
        src_u1, src_u2 = mk_src_u(u1), mk_src_u(u2)

        def mbc(stage, m):
            return mbc_t[:, stage * FROWS + m - 1:stage * FROWS + m]

        def mkc(m):
            return mkc_t[:, m - 1:m]

        for t in range(1, NPAIR + 5):
            if t <= NPAIR - 1:
                for dr in (2 * t + 2, 2 * t + 3):
                    if dr <= SR - 1:
                        nc.sync.dma_start(xr[0:64, dr % XRN, :], xin[:, dr, :])
                        nc.sync.dma_start(xr[64:128, dr % XRN, :],
                                          xin[:, SOFF + dr, :])
            # stage 0: conv1(x) -> T
            if 1 <= t <= NPAIR:
                m = 2 * t - 1
                ps = psum_tile(ppF, "psF")
                mm_pair(ps, lambda k: wcv_ap(0, k), src_xr, m)
                act2(T[:, m, 1:1 + W], T[:, m + 1, 1:1 + W], ps, AF.Identity,
                     mbc(0, m), mkc(m), mbc(0, m + 1), mkc(m + 1))
            # stage 1: relu(conv(T1)) -> u1
            pr = t - 1
            if 1 <= pr <= NPAIR:
                m = 2 * pr - 1
                ps = psum_tile(ppF, "psF")
                mm_pair(ps, lambda k: wcv_ap(1, k), src_T, m)
                act2(u1[:, m % URN, 1:1 + W], u1[:, (m + 1) % URN, 1:1 + W],
                     ps, AF.Relu, mbc(1, m), mkc(m), mbc(1, m + 1), mkc(m + 1))
            # stage 2: conv(u1) + T1 -> T (in place)
            pr = t - 2
            if 1 <= pr <= NPAIR:
                m = 2 * pr - 1
                ps = psum_tile(ppF, "psF")
                mm_pair(ps, lambda k: wcv_ap(2, k), src_u1, m)
                evict_resid_pair(T[:, m:m + 2, 1:1 + W], ps,
                                 mbc(2, m), mkc(m), mbc(2, m + 1), mkc(m + 1),
                                 nc.vector)
            # stage 3: relu(conv(T3)) -> u2
            pr = t - 3
            if 1 <= pr <= NPAIR:
                m = 2 * pr - 1
                ps = psum_tile(ppF, "psF")
                mm_pair(ps, lambda k: wcv_ap(3, k), src_T, m)
                act2(u2[:, m % URN, 1:1 + W], u2[:, (m + 1) % URN, 1:1 + W],
                     ps, AF.Relu, mbc(3, m), mkc(m), mbc(3, m + 1), mkc(m + 1))
            # stage 4: conv(u2) + T3 -> T (in place)
            pr = t - 4
            if 1 <= pr <= NPAIR:
                m = 2 * pr - 1
                ps = psum_tile(ppF, "psF")
                mm_pair(ps, lambda k: wcv_ap(4, k), src_u2, m)
                evict_resid_pair(T[:, m:m + 2, 1:1 + W], ps,
                                 mbc(4, m), mkc(m), mbc(4, m + 1), mkc(m + 1),
                                 nc.gpsimd)

        # ---------------- l15: 64 -> 32 into quarter tile (fp32, R=2) ------
        XQ = qp.tile([128, QR, WP], f32r, tag="XQ")
        nc.gpsimd.memset(XQ[:, :, 0:1].bitcast(f32), 0.0)
        nc.gpsimd.memset(XQ[:, :, WP - 1:WP].bitcast(f32), 0.0)
        Tf = T[:].bitcast(f32)
        for pj in range(QR // 2):
            dj = 2 * pj
            ps = psum_tile(ppF, "psF")
            for g in range(4):
                j = QOFF[g] + dj
                s = 0 if j + 1 <= 45 else 1
                rl = j - (0 if s == 0 else SOFF)
                for k, (r, c) in enumerate(RC):
                    nc.tensor.matmul(
                        ps[32 * g:32 * g + 32, :],
                        w15_t[64 * s:64 * s + 64, k * 32:(k + 1) * 32],
                        Tf[64 * s:64 * s + 64, rl + r:rl + r + 2,
                           1 + c:1 + c + W],
                        start=(k == 0), stop=(k == 8),
                        tile_position=(64 * s, 32 * g), skip_group_check=True)
            for g in range(4):
                for h in range(2):
                    nc.scalar.activation(
                        XQ[32 * g:32 * g + 32, dj + h, 1:1 + W],
                        ps[32 * g:32 * g + 32, 256 * h:256 * h + 256],
                        AF.Identity,
                        bias=mb15_t[32 * g:32 * g + 32, dj + h:dj + h + 1],
                        scale=mq15_t[32 * g:32 * g + 32, dj + h:dj + h + 1])

        if DBG == 'l15':
            for dj in range(QR):
                dt_ = sp.tile([128, 512], f32, tag="ev", name="dt_")
                nc.scalar.copy(dt_[:, 0:W], XQ[:, dj, 1:1 + W])
                nc.sync.dma_start(dbg[:, dj, :], dt_[:, 0:W])

        # ---------------- deform conv pairs (v2 separable scheme) ----------
        # Product-chain tiles use a merged (row,col) layout: a 2-row pair is
        # one contiguous 516 (padded) / 512 (out) span, so every bf16 engine
        # store is >=1KB contiguous.  The row seam at merged idx 257/258 of
        # Dh is never read.
        W2, WP2 = 2 * W, 2 * WP

        def rslot(row):
            # relu-ring slot for rows 1..26; 0/27 -> permanent-zero slot RRN
            if row < 1 or row > NSTEP:
                return RRN
            return (row - 1) % RRN

        def flat2(ap):
            return ap.rearrange("p a b -> p (a b)")

        def deform_step(d, p, fb_t, ring_src, ring_dst):
            """One 2-row step (out rows 2p-1, 2p) of deform conv d.
            Horizontal-first exact bilinear:
              h_r(j,i) = X(j+r,1+i) + relu(dx)*DX(j+r,1+i) + min(dx,0)*DX(j+r,i)
              s(j,i)   = h_0 + relu(dy)*(h_1-h_0) + min(dy,0)*(h_0-h_-1)
            with DX(y,x) = X(y,x+1)-X(y,x) shared across k and r.  All ops
            are <=3D (partition + 2 free dims): per-step repack tiles DXR/
            DXL/XR hold the needed 256-wide windows of the 4 source rows."""
            m = 2 * p - 1
            if ring_src is None:
                SRC, s0 = XQ, m - 1            # rows m-1..m+2 at s0..s0+3
            else:
                RS4 = gp.tile([128, 4, WP], f32r, tag="RS4", bufs=1)
                for i in range(4):
                    sl = rslot(m - 1 + i)
                    nc.gpsimd.tensor_copy(RS4[:, i:i + 1, :],
                                          ring_src[:, sl:sl + 1, :])
                SRC, s0 = RS4, 0
            # repacked windows of the 4 source rows (shared across k, r)
            DXR = gp.tile([128, 4, W], bf16, tag="DXR", bufs=1)
            nc.gpsimd.tensor_tensor(DXR[:], SRC[:, s0:s0 + 4, 2:2 + W],
                                    SRC[:, s0:s0 + 4, 1:1 + W], ALU.subtract)
            DXL = gp.tile([128, 4, W], bf16, tag="DXL", bufs=1)
            nc.gpsimd.tensor_tensor(DXL[:], SRC[:, s0:s0 + 4, 1:1 + W],
                                    SRC[:, s0:s0 + 4, 0:W], ALU.subtract)
            Dvt = gp.tile([128, 3, W], bf16, tag="Dvt", bufs=1)
            nc.gpsimd.tensor_tensor(Dvt[:], SRC[:, s0 + 1:s0 + 4, 1:1 + W],
                                    SRC[:, s0:s0 + 3, 1:1 + W], ALU.subtract)
            ps = psum_tile(ppD, "psD")
            W2 = 2 * W
            # X-term via the PE: ps = (sum_k W_k) @ X(m..m+1) starts the
            # accumulation group (h-planes below exclude X)
            nc.tensor.matmul(ps[:, 0:512], wds_t[:, d * 128:(d + 1) * 128],
                             SRC[:, s0 + 1:s0 + 3, 1:1 + W],
                             start=True, stop=False, skip_group_check=True)
            for g3 in range(3):
                kk = slice(3 * g3, 3 * g3 + 3)
                dyf = fb_t[:, kk, 0:W2]
                dxf = fb_t[:, kk, W2:1024]
                hs = []
                for ri, r in enumerate((-1, 0, 1)):
                    DRb = flat2(DXR[:, r + 1:r + 3, :]).unsqueeze(
                        1).broadcast_to([128, 3, W2])
                    DLb = flat2(DXL[:, r + 1:r + 3, :]).unsqueeze(
                        1).broadcast_to([128, 3, W2])
                    tG = gp.tile([128, 3, W2], bf16, tag="tG", bufs=2)
                    nc.vector.scalar_tensor_tensor(tG[:], dxf, 0.0, DRb,
                                                   ALU.max, ALU.mult)
                    tH = gp.tile([128, 3, W2], bf16, tag="tH", bufs=2)
                    nc.vector.scalar_tensor_tensor(tH[:], dxf, 0.0, DLb,
                                                   ALU.min, ALU.mult)
                    hp = gp.tile([128, 3, W2], bf16, tag=f"h{ri}",
                                 name=f"h{ri}")
                    nc.gpsimd.tensor_tensor(hp[:], tG[:], tH[:], ALU.add)
                    hs.append(hp)
                h0, h1, h2 = hs
                DvA = flat2(Dvt[:, 1:3, :]).unsqueeze(1).broadcast_to(
                    [128, 3, W2])
                DvB = flat2(Dvt[:, 0:2, :]).unsqueeze(1).broadcast_to(
                    [128, 3, W2])
                dhA = gp.tile([128, 3, W2], bf16, tag="tG", name="dhA", bufs=2)
                nc.vector.tensor_tensor(dhA[:], h2[:], h1[:], ALU.subtract)
                nc.vector.tensor_tensor(dhA[:], dhA[:], DvA, ALU.add)
                dhB = gp.tile([128, 3, W2], bf16, tag="tH", name="dhB", bufs=2)
                nc.gpsimd.tensor_tensor(dhB[:], h1[:], h0[:], ALU.subtract)
                nc.vector.tensor_tensor(dhB[:], dhB[:], DvB, ALU.add)
                vG = gp.tile([128, 3, W2], bf16, tag="h2", name="vG")
                nc.vector.scalar_tensor_tensor(vG[:], dyf, 0.0, dhA[:],
                                               ALU.max, ALU.mult)
                vH = gp.tile([128, 3, W2], bf16, tag="h0", name="vH")
                nc.vector.scalar_tensor_tensor(vH[:], dyf, 0.0, dhB[:],
                                               ALU.min, ALU.mult)
                st = gp.tile([128, 3, W2], bf16, tag="st", bufs=1)
                nc.gpsimd.tensor_tensor(st[:], vG[:], vH[:], ALU.add)
                nc.gpsimd.tensor_tensor(st[:], st[:], h1[:], ALU.add)
                for i in range(3):
                    k = 3 * g3 + i
                    nc.tensor.matmul(ps[:, 0:512], wdf_ap(d, k),
                                     st[:, i, :],
                                     start=False, stop=(k == 8),
                                     skip_group_check=True)
            mb0 = mbq_t[:, d * NSTEP + m - 1:d * NSTEP + m]
            mb1 = mbq_t[:, d * NSTEP + m:d * NSTEP + m + 1]
            mk0, mk1 = mkq_t[:, m - 1:m], mkq_t[:, m:m + 1]
            if ring_dst is not None:
                act2(ring_dst[:, rslot(m), 1:1 + W],
                     ring_dst[:, rslot(m + 1), 1:1 + W],
                     ps, AF.Relu, mb0, mk0, mb1, mk1)
            else:
                evict_resid_pair(XQ[:, m:m + 2, 1:1 + W], ps,
                                 mb0, mk0, mb1, mk1, nc.vector)

        def shuffled_fields(p):
            fc_t = fr.tile([128, 1024], bf16, tag="fc", bufs=1)
            nc.sync.dma_start(fc_t[:], fcp[p - 1])
            fb_t = fr.tile([128, 9, 1024], bf16, tag="fb", bufs=1)
            for k in range(9):
                nc.vector.stream_shuffle(fb_t[:, k, :], fc_t[:], [k] * 32)
            return fb_t

        def deform_round(d_fwd, d_bwd, ring):
            for p in range(1, NP + 2):
                if p <= NP:
                    deform_step(d_fwd, p, shuffled_fields(p), None, ring)
                if p >= 2:
                    deform_step(d_bwd, p - 1, shuffled_fields(p - 1), ring,
                                None)

        if DBG == 'd50':
            pass  # placeholder
        r5 = qp.tile([128, RRN + 1, WP], f32r, tag="r5")
        nc.gpsimd.memset(r5[:].bitcast(f32), 0.0)
        deform_round(0, 1, r5)
        r6 = qp.tile([128, RRN + 1, WP], f32r, tag="r6")
        nc.gpsimd.memset(r6[:].bitcast(f32), 0.0)
        deform_round(2, 3, r6)

        # ---------------- l24: 32 -> 3 on the final 64 rows (fp32, R=2) ----
        XQf = XQ[:].bitcast(f32)
        for pj in range(32):
            jo = 2 * pj
            j = jo + HALO
            g = min(jo // 16, 3)
            dj = j - QOFF[g]
            ps = psum_tile(ppF, "psF")
            for k, (r, c) in enumerate(RC):
                nc.tensor.matmul(
                    ps[0:3, :], w24_t[32 * g:32 * g + 32, k * 3:(k + 1) * 3],
                    XQf[32 * g:32 * g + 32, dj + r:dj + r + 2,
                        1 + c:1 + c + W],
                    start=(k == 0), stop=(k == 8), tile_position=(32 * g, 0),
                    skip_group_check=True)
            ob = sp.tile([128, 512], f32, tag="ev")
            nc.scalar.activation(ob[0:3, 0:256], ps[0:3, 0:256], AF.Identity,
                                 bias=b24_t[0:3, :])
            nc.scalar.activation(ob[0:3, 256:512], ps[0:3, 256:512],
                                 AF.Identity, bias=b24_t[0:3, :])
            nc.sync.dma_start(
                out[:, jo:jo + 2, :],
                ob[0:3, :].rearrange("p (a b) -> p a b", a=2))

    import concourse.mybir as mybir2
    split_excess_waits(nc, mybir2)
    return nc


# ----------------------------------------------------------------------------
# host side
# ----------------------------------------------------------------------------
def _bd_front(w):
    """[64,64,3,3] -> [9, 128, 128] block-diag lhsT (slab0+slab1)."""
    o = np.zeros((9, 128, 128), np.float32)
    for k, (r, c) in enumerate(RC):
        l = np.ascontiguousarray(w[:, :, r + 1, c + 1].T)   # [in, out]
        o[k, 0:64, 0:64] = l
        o[k, 64:128, 64:128] = l
    return o


def _bd_deform(w):
    """[32,32,3,3] -> [9, 128, 128] 4-quarter block-diag lhsT (grid order)."""
    o = np.zeros((9, 128, 128), np.float32)
    for k in range(9):
        l = w[:, :, GY[k] + 1, GX[k] + 1].T
        for g in range(4):
            o[k, 32 * g:32 * g + 32, 32 * g:32 * g + 32] = l
    return o


def _lhsT_dup2(w, co):
    o = np.empty((9, 128, co), np.float32)
    for k, (r, c) in enumerate(RC):
        l = np.ascontiguousarray(w[:, :, r + 1, c + 1].T)
        o[k, 0:64] = l
        o[k, 64:128] = l
    return o


def _lhsT_dup4(w, co, grid=False):
    o = np.empty((9, 128, co), np.float32)
    for k in range(9):
        if grid:
            l = w[:, :, GY[k] + 1, GX[k] + 1].T
        else:
            r, c = RC[k]
            l = w[:, :, r + 1, c + 1].T
        for g in range(4):
            o[k, 32 * g:32 * g + 32] = l
    return o


def _flat_w(stack):
    """[S, 9, 128, co] or [9, 128, co] -> [128, S*9*co]"""
    a = np.asarray(stack, np.float32)
    if a.ndim == 3:
        a = a[None]
    return np.ascontiguousarray(a.transpose(2, 0, 1, 3).reshape(128, -1))


def _strip(a, r0, rows):
    C, H, _ = a.shape
    t = np.zeros((C, rows, WP), np.float32)
    lo, hi = max(r0, 0), min(r0 + rows, H)
    if hi > lo:
        t[:, lo - r0:hi - r0, 1:1 + W] = a[:, lo:hi]
    return t


def _prep_shards(inputs):
    x = np.asarray(inputs['x'], np.float32)
    off = np.asarray(inputs['offset_0'], np.float32)
    B, C, H, Wi = x.shape

    wcv = _flat_w(np.stack([_bd_front(np.asarray(inputs[n], np.float32))
                            for n in ('l12_w', 'l13_w1', 'l13_w2',
                                      'l14_w1', 'l14_w2')]))
    w15a = _flat_w(_lhsT_dup2(np.asarray(inputs['l15_w'], np.float32), 32))
    wdf_s = np.stack([_bd_deform(np.asarray(inputs[n], np.float32))
                      for n in ('d50_w', 'd51_w', 'd60_w', 'd61_w')])
    wdf = _flat_w(wdf_s).astype(BF16)
    wds = np.ascontiguousarray(
        wdf_s.sum(axis=1).transpose(1, 0, 2).reshape(128, -1))
    w24a = _flat_w(_lhsT_dup4(np.asarray(inputs['l24_w'], np.float32), 3))
    b24 = np.zeros((128, 1), np.float32)
    b24[0:3, 0] = np.asarray(inputs['l24_b'], np.float32)

    fb = {k: np.asarray(inputs[k], np.float32) for k in
          ('l12_b', 'l13_b1', 'l13_b2', 'l14_b1', 'l14_b2', 'l15_b',
           'd50_b', 'd51_b', 'd60_b', 'd61_b')}

    shards = []
    for b in range(B):
        ov = off[b].reshape(12, 2, H, Wi)
        crop = ov[3:12]
        dxs_f = crop[:, 0] * RATIO
        dys_f = crop[:, 1] * RATIO
        for g4 in range(4):
            r0 = g4 * 64 - HALO

            def m(sr):
                return np.float32(1.0 if 0 <= r0 + sr < H else 0.0)

            xin = _strip(x[b], r0, ROWS)

            dxs = _strip(dxs_f, r0, ROWS)    # [9, ROWS, WP]
            dys = _strip(dys_f, r0, ROWS)
            fc = np.zeros((NP, 128, 1024), np.float32)
            for g in range(4):
                rows = QOFF[g] + np.arange(1, NSTEP + 1)
                dyp = dys[:, rows, 1:1 + W].reshape(9, NP, 2 * W)
                dxp = dxs[:, rows, 1:1 + W].reshape(9, NP, 2 * W)
                for k in range(9):
                    fc[:, 32 * g + k, 0:2 * W] = dyp[k]
                    fc[:, 32 * g + k, 2 * W:1024] = dxp[k]

            mkc = np.zeros((128, FROWS), np.float32)
            for i2 in range(1, FROWS + 1):
                mkc[0:64, i2 - 1] = m(i2)
                mkc[64:128, i2 - 1] = m(SOFF + i2)
            mbc = np.zeros((128, 5 * FROWS), np.float32)
            for si, nm in enumerate(('l12_b', 'l13_b1', 'l13_b2',
                                     'l14_b1', 'l14_b2')):
                col = np.concatenate([fb[nm], fb[nm]])
                mbc[:, si * FROWS:(si + 1) * FROWS] = mkc * col[:, None]
            mq = np.zeros((128, QR), np.float32)
            for dj in range(QR):
                for qg in range(4):
                    mq[32 * qg:32 * qg + 32, dj] = m(QOFF[qg] + dj)
            mb15v = mq * np.tile(fb['l15_b'], 4)[:, None]
            mkq = np.zeros((128, NSTEP), np.float32)
            for jj in range(NSTEP):
                for qg in range(4):
                    mkq[32 * qg:32 * qg + 32, jj] = m(QOFF[qg] + 1 + jj)
            mbq = np.zeros((128, 4 * NSTEP), np.float32)
            for di, nm in enumerate(('d50_b', 'd51_b', 'd60_b', 'd61_b')):
                mbq[:, di * NSTEP:(di + 1) * NSTEP] = \
                    mkq * np.tile(fb[nm], 4)[:, None]

            shards.append({
                'xin': xin, 'fcp': fc.astype(BF16), 'wcv': wcv, 'w15': w15a,
                'wdf': wdf, 'wds': wds, 'w24': w24a, 'b24': b24,
                'maskc': mkc,
                'mbiasc': mbc, 'mq15': mq, 'mb15': mb15v, 'maskq': mkq,
                'mbiasq': mbq,
            })
    return shards


def kernel(**inputs):
    if 'nc' not in _cache:
        _cache['nc'] = build_nc()
    from concourse.bass_utils import run_bass_kernel_spmd
    shards = _prep_shards(inputs)
    res = run_bass_kernel_spmd(_cache['nc'], shards, core_ids=list(range(8)))
    out = np.empty((2, 3, 256, 256), np.float32)
    for i in range(8):
        b, g = divmod(i, 4)
        out[b, :, g * 64:(g + 1) * 64, :] = res.results[i]['out']
    return out
